# revision 31
# baseline (speedup 1.0000x reference)
"""Trainium2 Bass kernel for nn_CapLayerLP: box+cap+fairness QP via
primal-dual predictor-corrector interior point.

v2 vs the original baseline (359us):
- Warm start s0=z0=2.0 (residual scheme generalized to rp_p0 != 0):
  converges in 8 iterations instead of 16 (validated in fp32 sim with
  noise-perturbation robustness checks; rel err ~2e-4 vs 2e-2 gate).
- 3-direction decomposition: the Newton direction map is affine in the
  complementarity residual rsz, so the corrector is assembled as
  d_c = d_affine + B(ds_a*dz_a) + nsmu*B(1), where B (linear part, no
  rp/rx terms) is computed OFF the critical path: B(1) during stage A,
  B(pq) during the affine step-length PE round trips.
- mu_aff via quadratic expansion mu_aff*m = mu*m + a*c1 + a^2*c2
  (coefficients reduced before alpha is known -> no post-alpha
  reduction round trip).
- Uniform DSZ=[ds|dz] layout -> one fused ratio tile for the step
  length and one fused s/z update.
- Narrow scalar algebra spread onto GpSimd + Activation engines so the
  DVE critical path stays on the wide tiles.

Sharding: batch is 1 and the solve is latency-bound; replicated on all
8 cores, core 0's output returned.
"""
import os

import numpy as np

import concourse.bass as bass
import concourse.bacc as bacc
import concourse.tile as tile
from concourse import mybir
from concourse.bass_utils import run_bass_kernel_spmd

AL = mybir.AluOpType
F32 = mybir.dt.float32
AX = mybir.AxisListType.X

N = 1024
P = 128
CO = N // P            # 8
V = 2 * CO             # 16
NS = V + 3             # 19
C_CAP = 10.0
EPS = 1e-4
INIT = 2.0             # warm-start scale for s and z
ITERS = int(os.environ.get("KD_ITERS", "8"))
M_CONST = 2 * N + 3
CLAMP = 1e-30
TINY = 1e-12


def _build(nc: bass.Bass):
    x_d = nc.dram_tensor("x", [1, N], F32, kind="ExternalInput")
    f_d = nc.dram_tensor("ind", [N], mybir.dt.int32, kind="ExternalInput")
    ones_d = nc.dram_tensor("ones", [P, P], F32, kind="ExternalInput")
    ident_d = nc.dram_tensor("ident", [P, P], F32, kind="ExternalInput")
    out_d = nc.dram_tensor("out", [1, N], F32, kind="ExternalOutput")

    x_ap = x_d[:, :].rearrange("a (p c) -> a p c", p=P)[0]
    f_ap = f_d[:].rearrange("(p c) -> p c", p=P)
    o_ap = out_d[:, :].rearrange("a (p c) -> a p c", p=P)[0]

    with tile.TileContext(nc) as tc:
        with (
            tc.tile_pool(name="const", bufs=1) as cns,
            tc.tile_pool(name="state", bufs=1) as st,
            tc.tile_pool(name="scr", bufs=2) as sc,
            tc.tile_pool(name="psum", bufs=2, space="PSUM") as ps,
            tc.tile_pool(name="psum1", bufs=2, space="PSUM") as ps1,
            tc.tile_pool(name="psumq", bufs=2, space="PSUM") as psq,
        ):
            # ---------------- setup ----------------
            ONES = cns.tile([P, P], F32)
            IDENT = cns.tile([P, P], F32)
            nc.sync.dma_start(out=ONES[:, :], in_=ones_d[:, :])
            nc.sync.dma_start(out=IDENT[:, :], in_=ident_d[:, :])

            F8 = cns.tile([P, CO], F32)
            nc.gpsimd.dma_start(out=F8, in_=f_ap)  # int32 -> f32 cast

            XIN = cns.tile([P, CO], F32)
            nc.sync.dma_start(out=XIN, in_=x_ap)
            RX0 = cns.tile([P, CO], F32)           # rx0 = INIT - x_in
            nc.vector.tensor_scalar(out=RX0, in0=XIN, scalar1=-1.0,
                                    scalar2=INIT, op0=AL.mult, op1=AL.add)

            XT = st.tile([P, CO], F32)
            nc.vector.memset(XT, 0.0)
            SZ = st.tile([P, 2 * NS], F32)
            nc.vector.memset(SZ, INIT)
            # PH = [phi | 2phi | -phi | -2phi | phi*RF(3) | -phi*RF2]
            PH = st.tile([P, 8], F32)
            nc.vector.memset(PH[:, 0:1], 1.0)
            nc.vector.memset(PH[:, 1:2], 2.0)
            nc.vector.memset(PH[:, 2:3], -1.0)
            nc.vector.memset(PH[:, 3:4], -2.0)
            PHI = PH[:, 0:1]
            PH2 = PH[:, 1:2]
            NPHI = PH[:, 2:3]
            NPHI2 = PH[:, 3:4]

            facc = sc.tile([P, 1], F32, tag="facc")
            nc.vector.reduce_sum(facc, F8, axis=AX)
            NMp = ps.tile([P, 1], F32, tag="pscr")
            nc.tensor.matmul(NMp, ONES, facc)      # Nm replicated

            # rp0 scalars into PH[4:8]: [INIT-C, INIT-C*Nm/N-1,
            # INIT+C*Nm/N, -(INIT+C*Nm/N)] (scaled by phi each iter)
            nc.vector.memset(PH[:, 4:5], INIT - C_CAP)
            nc.vector.tensor_scalar(out=PH[:, 5:6], in0=NMp,
                                    scalar1=-C_CAP / N, scalar2=INIT - 1.0,
                                    op0=AL.mult, op1=AL.add)
            nc.vector.tensor_scalar(out=PH[:, 6:7], in0=NMp,
                                    scalar1=C_CAP / N, scalar2=INIT,
                                    op0=AL.mult, op1=AL.add)
            nc.vector.tensor_scalar(out=PH[:, 7:8], in0=NMp,
                                    scalar1=-C_CAP / N, scalar2=-INIT,
                                    op0=AL.mult, op1=AL.add)
            HFA = cns.tile([P, 1], F32)            # C*Nm/N + 1 (= hf1)
            nc.vector.tensor_scalar(out=HFA, in0=NMp, scalar1=C_CAP / N,
                                    scalar2=1.0, op0=AL.mult, op1=AL.add)
            HFB = cns.tile([P, 1], F32)            # C*Nm/N (= -hf2)
            nc.vector.tensor_scalar(out=HFB, in0=NMp, scalar1=C_CAP / N,
                                    scalar2=None, op0=AL.mult)

            AIBI = st.tile([P, 3], F32)            # [AINV | BINV | ApSd]

            s_v = SZ[:, 0:V]
            s_s = SZ[:, V:NS]
            z_v = SZ[:, NS:NS + V]
            z_s = SZ[:, NS + V:2 * NS]

            # ---------------- iterations ----------------
            for it in range(ITERS):
                RPs = PH[:, 4:7]
                NRPs2 = PH[:, 7:8]
                # --- [V] stage A wide chain ---
                R = sc.tile([P, 2 * NS], F32, tag="R")
                nc.vector.reciprocal(R, SZ)
                W = sc.tile([P, NS], F32, tag="W")
                nc.vector.tensor_tensor(out=W, in0=SZ[:, NS:2 * NS],
                                        in1=R[:, 0:NS], op=AL.mult)
                Dt = sc.tile([P, CO], F32, tag="Dt")
                nc.vector.scalar_tensor_tensor(
                    out=Dt, in0=W[:, 0:CO], scalar=EPS, in1=W[:, CO:V],
                    op0=AL.add, op1=AL.add)
                DI = sc.tile([P, CO], F32, tag="DI")
                nc.vector.reciprocal(DI, Dt)
                accA = sc.tile([P, 3], F32, tag="accA")  # [Sv|Su|mac]
                DIF = sc.tile([P, CO], F32, tag="DIF")
                nc.vector.scalar_tensor_tensor(
                    out=DIF, in0=DI, scalar=1.0, in1=F8,
                    op0=AL.bypass, op1=AL.mult, accum_out=accA[:, 0:1])
                nc.vector.reduce_sum(accA[:, 1:2], DI, axis=AX)
                SZA = sc.tile([P, NS], F32, tag="SZA")   # rsz_a = s*z
                nc.vector.scalar_tensor_tensor(
                    out=SZA[:, 0:V], in0=s_v, scalar=1.0, in1=z_v,
                    op0=AL.bypass, op1=AL.mult, accum_out=accA[:, 2:3])
                # --- [G] trio chains (SZA trio, NT, B(1) parts, Bt) ---
                nc.gpsimd.tensor_tensor(out=AIBI[:, 0:1],
                                        in0=SZ[:, V:V + 1],
                                        in1=R[:, NS + V:NS + V + 1],
                                        op=AL.mult)
                nc.gpsimd.tensor_tensor(out=SZA[:, V:NS], in0=s_s, in1=z_s,
                                        op=AL.mult)
                u_nt = sc.tile([P, 3], F32, tag="u_nt")
                nc.gpsimd.tensor_tensor(out=u_nt, in0=z_s, in1=RPs,
                                        op=AL.mult)
                v_nt = sc.tile([P, 3], F32, tag="v_nt")
                nc.gpsimd.tensor_tensor(out=v_nt, in0=SZA[:, V:NS],
                                        in1=u_nt, op=AL.subtract)
                NTa = sc.tile([P, 3], F32, tag="NTa")
                nc.gpsimd.tensor_tensor(out=NTa, in0=v_nt, in1=R[:, V:NS],
                                        op=AL.mult)
                NTDF = sc.tile([P, 1], F32, tag="NTDF")
                nc.gpsimd.tensor_tensor(out=NTDF, in0=NTa[:, 1:2],
                                        in1=NTa[:, 2:3], op=AL.subtract)
                Bt = sc.tile([P, 1], F32, tag="Bt")
                nc.gpsimd.tensor_tensor(out=Bt, in0=W[:, V + 1:V + 2],
                                        in1=W[:, V + 2:NS], op=AL.add)
                # --- [PE] MM1: stage-A sums ---
                VUS = ps.tile([P, 3], F32, tag="pscr")
                nc.tensor.matmul(VUS, ONES, accA)
                VUSS = sc.tile([P, 3], F32, tag="VUSS")
                nc.scalar.copy(VUSS, VUS)

                # --- [V] affine rhs chain ---
                tmr = sc.tile([P, CO], F32, tag="tmr")
                nc.vector.scalar_tensor_tensor(
                    out=tmr, in0=SZ[:, NS:NS + CO], scalar=PH2,
                    in1=SZA[:, 0:CO], op0=AL.mult, op1=AL.subtract)
                tm = sc.tile([P, CO], F32, tag="tm")
                nc.vector.tensor_tensor(out=tm, in0=tmr, in1=R[:, 0:CO],
                                        op=AL.mult)
                A1 = sc.tile([P, CO], F32, tag="A1")
                nc.vector.scalar_tensor_tensor(
                    out=A1, in0=RX0, scalar=NPHI, in1=tm,
                    op0=AL.mult, op1=AL.add)
                tppr = sc.tile([P, CO], F32, tag="tppr")
                nc.vector.scalar_tensor_tensor(
                    out=tppr, in0=SZ[:, NS + CO:NS + V], scalar=PHI,
                    in1=SZA[:, CO:V], op0=AL.mult, op1=AL.subtract)
                tppn = sc.tile([P, CO], F32, tag="tppn")
                nc.vector.tensor_tensor(out=tppn, in0=tppr,
                                        in1=R[:, CO:V], op=AL.mult)
                A2 = sc.tile([P, CO], F32, tag="A2")
                nc.vector.tensor_tensor(out=A2, in0=A1, in1=tppn,
                                        op=AL.subtract)
                B1a = sc.tile([P, CO], F32, tag="B1a")
                nc.vector.scalar_tensor_tensor(
                    out=B1a, in0=F8, scalar=NTDF, in1=A2,
                    op0=AL.mult, op1=AL.add)
                accS = sc.tile([P, 2], F32, tag="accS")
                Ya = sc.tile([P, CO], F32, tag="Ya")
                nc.vector.scalar_tensor_tensor(
                    out=Ya, in0=B1a, scalar=NTa[:, 0:1], in1=DI,
                    op0=AL.add, op1=AL.mult, accum_out=accS[:, 0:1])
                Yaf = sc.tile([P, CO], F32, tag="Yaf")
                nc.vector.scalar_tensor_tensor(
                    out=Yaf, in0=Ya, scalar=1.0, in1=F8,
                    op0=AL.bypass, op1=AL.mult, accum_out=accS[:, 1:2])
                # --- [PE] MM3 (S12a) ---
                S12a = ps.tile([P, 2], F32, tag="pscr")
                nc.tensor.matmul(S12a, ONES, accS)

                # --- [G] det chain (needs VUSS, BINV) ---
                Sd = sc.tile([P, 1], F32, tag="Sd")
                nc.gpsimd.tensor_tensor(out=Sd, in0=VUSS[:, 1:2],
                                        in1=VUSS[:, 0:1], op=AL.subtract)
                SpSd = sc.tile([P, 1], F32, tag="SpSd")
                nc.gpsimd.tensor_tensor(out=SpSd, in0=VUSS[:, 0:1],
                                        in1=Sd, op=AL.add)
                T3 = sc.tile([P, 1], F32, tag="T3")
                nc.gpsimd.tensor_tensor(out=T3, in0=VUSS[:, 0:1], in1=Sd,
                                        op=AL.mult)
                nc.gpsimd.tensor_tensor(out=AIBI[:, 2:3], in0=AIBI[:, 0:1],
                                        in1=Sd, op=AL.add)
                ApSd = AIBI[:, 2:3]
                M22t = sc.tile([P, 1], F32, tag="M22t")
                nc.gpsimd.tensor_tensor(out=M22t, in0=AIBI[:, 0:1],
                                        in1=VUSS[:, 0:1], op=AL.mult)

                # --- [V] BINV ---
                nc.vector.reciprocal(AIBI[:, 1:2], Bt)

                # --- [G] finish det: det = AINV*Sv + BINV*(AINV+SpSd*?) ---
                # det = AINV*(BINV + Sv) + BINV*(Sv + Sd) + Sv*Sd
                #     = AINV*BINV + AINV*Sv(=M22t) ... use:
                # T2 = BINV*(AINV + SpSd) = AINV*BINV + BINV*(Sv+Sd)
                ApSp = sc.tile([P, 1], F32, tag="ApSp")
                nc.gpsimd.tensor_tensor(out=ApSp, in0=AIBI[:, 0:1],
                                        in1=SpSd, op=AL.add)
                T2 = sc.tile([P, 1], F32, tag="T2")
                nc.gpsimd.tensor_tensor(out=T2, in0=AIBI[:, 1:2],
                                        in1=ApSp, op=AL.mult)
                qb = sc.tile([P, 1], F32, tag="qb")
                nc.gpsimd.tensor_tensor(out=qb, in0=T2, in1=T3, op=AL.add)
                DETt = sc.tile([P, 1], F32, tag="DETt")
                nc.gpsimd.tensor_tensor(out=DETt, in0=M22t, in1=qb,
                                        op=AL.add)
                DETI = sc.tile([P, 1], F32, tag="DETI")
                nc.vector.reciprocal(DETI, DETt)
                msca = sc.tile([P, 1], F32, tag="msca")
                nc.gpsimd.tensor_tensor(out=msca, in0=SZA[:, V:V + 1],
                                        in1=SZA[:, V + 1:V + 2], op=AL.add)
                mscb = sc.tile([P, 1], F32, tag="mscb")
                nc.gpsimd.tensor_tensor(out=mscb, in0=msca,
                                        in1=SZA[:, V + 2:NS], op=AL.add)
                # B(1): TB = R_s trio; q1 = Rp - Rm
                TBdf1 = sc.tile([P, 1], F32, tag="TBdf1")
                nc.gpsimd.tensor_tensor(out=TBdf1, in0=R[:, V + 1:V + 2],
                                        in1=R[:, V + 2:NS], op=AL.subtract)
                q1B = sc.tile([P, CO], F32, tag="q1B")
                nc.gpsimd.tensor_tensor(out=q1B, in0=R[:, CO:V],
                                        in1=R[:, 0:CO], op=AL.subtract)
                MUm = sc.tile([P, 1], F32, tag="MUm")
                nc.vector.tensor_tensor(out=MUm, in0=mscb,
                                        in1=VUSS[:, 2:3], op=AL.add)
                mui = sc.tile([P, 1], F32, tag="mui")
                nc.vector.reciprocal(mui, MUm)
                K = sc.tile([P, 1], F32, tag="K")
                nc.vector.tensor_scalar(out=K, in0=mui, scalar1=mui,
                                        scalar2=-1.0 / M_CONST,
                                        op0=AL.mult, op1=AL.mult)

                # --- [A] negated psum copies ---
                S12s = sc.tile([P, 2], F32, tag="S12s")
                nc.scalar.mul(S12s, S12a, -1.0)

                # --- [V] albe_a (negated) + dx_a ---
                S1m2a = sc.tile([P, 1], F32, tag="S1m2a")
                nc.vector.tensor_tensor(out=S1m2a, in0=S12s[:, 0:1],
                                        in1=S12s[:, 1:2], op=AL.subtract)
                q2a = sc.tile([P, 1], F32, tag="q2a")
                nc.vector.tensor_tensor(out=q2a, in0=S1m2a,
                                        in1=VUSS[:, 0:1], op=AL.mult)
                ABa = sc.tile([P, 2], F32, tag="ABa")
                nc.vector.tensor_scalar(out=ABa[:, 0:1], in0=AIBI[:, 1:2],
                                        scalar1=S12s[:, 0:1], scalar2=q2a,
                                        op0=AL.mult, op1=AL.add)
                nc.vector.tensor_scalar(out=ABa[:, 1:2], in0=ApSd,
                                        scalar1=S12s[:, 1:2], scalar2=q2a,
                                        op0=AL.mult, op1=AL.subtract)
                albe_an = sc.tile([P, 2], F32, tag="albe_an")
                nc.vector.tensor_scalar(out=albe_an, in0=ABa,
                                        scalar1=DETI, scalar2=None,
                                        op0=AL.mult)
                t1a = sc.tile([P, CO], F32, tag="t1a")
                nc.vector.scalar_tensor_tensor(
                    out=t1a, in0=DI, scalar=albe_an[:, 0:1], in1=Ya,
                    op0=AL.mult, op1=AL.add)
                DXa = sc.tile([P, CO], F32, tag="DXa")
                nc.vector.scalar_tensor_tensor(
                    out=DXa, in0=DIF, scalar=albe_an[:, 1:2], in1=t1a,
                    op0=AL.mult, op1=AL.add)

                # --- ds_a: [V] m/p blocks, [G] trio ---
                DSZa = sc.tile([P, 2 * NS], F32, tag="DSZa")
                nc.vector.tensor_scalar(out=DSZa[:, 0:CO], in0=DXa,
                                        scalar1=NPHI2, scalar2=None,
                                        op0=AL.add)
                nc.vector.tensor_scalar(out=DSZa[:, CO:V], in0=DXa,
                                        scalar1=-1.0, scalar2=NPHI,
                                        op0=AL.mult, op1=AL.add)
                SFPa = sc.tile([P, 2], F32, tag="SFPa")
                nc.gpsimd.tensor_tensor(out=SFPa, in0=AIBI[:, 0:2],
                                        in1=albe_an, op=AL.mult)
                nc.gpsimd.tensor_tensor(out=DSZa[:, V:V + 2],
                                        in0=SFPa, in1=RPs[:, 0:2],
                                        op=AL.subtract)
                nc.gpsimd.tensor_tensor(out=DSZa[:, V + 2:NS], in0=NRPs2,
                                        in1=SFPa[:, 1:2], op=AL.subtract)

                # --- [V] dz_a (19-wide incl trio) + steplen-a ---
                uva = sc.tile([P, NS], F32, tag="uva")
                nc.vector.tensor_tensor(out=uva, in0=SZ[:, NS:2 * NS],
                                        in1=DSZa[:, 0:NS], op=AL.mult)
                vva = sc.tile([P, NS], F32, tag="vva")
                nc.vector.tensor_tensor(out=vva, in0=uva, in1=SZA,
                                        op=AL.add)
                nc.vector.scalar_tensor_tensor(
                    out=DSZa[:, NS:2 * NS], in0=vva, scalar=-1.0,
                    in1=R[:, 0:NS], op0=AL.mult, op1=AL.mult)
                Qa = sc.tile([P, 2 * NS], F32, tag="Qa")
                nc.vector.scalar_tensor_tensor(
                    out=Qa, in0=DSZa, scalar=-1.0, in1=R,
                    op0=AL.mult, op1=AL.mult)
                qpa = sc.tile([P, 1], F32, tag="qpa")
                nc.vector.reduce_max(qpa, Qa, axis=AX)

                # --- [PE] T-a ---
                qrow_a = psq.tile([1, P], F32, tag="qrow")
                nc.tensor.transpose(qrow_a, qpa, IDENT)

                # --- [V] pq + mu_aff coefficients (alpha-independent) ---
                PQ = sc.tile([P, NS], F32, tag="PQ")
                accC = sc.tile([P, 2], F32, tag="accC")  # [c1v|c2v]
                nc.vector.scalar_tensor_tensor(
                    out=PQ[:, 0:V], in0=DSZa[:, 0:V], scalar=1.0,
                    in1=DSZa[:, NS:NS + V], op0=AL.bypass, op1=AL.mult,
                    accum_out=accC[:, 1:2])
                cz1 = sc.tile([P, V], F32, tag="cz1")
                nc.vector.tensor_tensor(out=cz1, in0=SZ[:, 0:V],
                                        in1=DSZa[:, NS:NS + V], op=AL.mult)
                czt = sc.tile([P, V], F32, tag="czt")
                nc.vector.scalar_tensor_tensor(
                    out=czt, in0=cz1, scalar=1.0, in1=uva[:, 0:V],
                    op0=AL.bypass, op1=AL.add, accum_out=accC[:, 0:1])
                # B(1) y-parts on V (T-a window), then MM2
                q2B = sc.tile([P, CO], F32, tag="q2B")
                nc.vector.scalar_tensor_tensor(
                    out=q2B, in0=F8, scalar=TBdf1, in1=q1B,
                    op0=AL.mult, op1=AL.add)
                accB = sc.tile([P, 2], F32, tag="accB")
                yB1 = sc.tile([P, CO], F32, tag="yB1")
                nc.vector.scalar_tensor_tensor(
                    out=yB1, in0=q2B, scalar=R[:, V:V + 1], in1=DI,
                    op0=AL.add, op1=AL.mult, accum_out=accB[:, 0:1])
                yB1f = sc.tile([P, CO], F32, tag="yB1f")
                nc.vector.scalar_tensor_tensor(
                    out=yB1f, in0=yB1, scalar=1.0, in1=F8,
                    op0=AL.bypass, op1=AL.mult, accum_out=accB[:, 1:2])
                VB = ps.tile([P, 2], F32, tag="pscr")
                nc.tensor.matmul(VB, ONES, accB)

                # --- [G] trio pq + cz sums ---
                nc.gpsimd.tensor_tensor(out=PQ[:, V:NS],
                                        in0=DSZa[:, V:NS],
                                        in1=DSZa[:, NS + V:2 * NS],
                                        op=AL.mult)
                czs1 = sc.tile([P, 3], F32, tag="czs1")
                nc.gpsimd.tensor_tensor(out=czs1, in0=s_s,
                                        in1=DSZa[:, NS + V:2 * NS],
                                        op=AL.mult)
                czs = sc.tile([P, 3], F32, tag="czs")
                nc.gpsimd.tensor_tensor(out=czs, in0=czs1,
                                        in1=uva[:, V:NS], op=AL.add)
                c1sa = sc.tile([P, 1], F32, tag="c1sa")
                nc.gpsimd.tensor_tensor(out=c1sa, in0=czs[:, 0:1],
                                        in1=czs[:, 1:2], op=AL.add)
                c2sa = sc.tile([P, 1], F32, tag="c2sa")
                nc.gpsimd.tensor_tensor(out=c2sa, in0=PQ[:, V:V + 1],
                                        in1=PQ[:, V + 1:V + 2], op=AL.add)
                c12s = sc.tile([P, 2], F32, tag="c12s")
                nc.gpsimd.tensor_tensor(out=c12s[:, 0:1], in0=c1sa,
                                        in1=czs[:, 2:3], op=AL.add)
                nc.gpsimd.tensor_tensor(out=c12s[:, 1:2], in0=c2sa,
                                        in1=PQ[:, V + 2:NS], op=AL.add)

                # --- [G] B(pq) prep (fills the T-a PE window) ---
                TBpq = sc.tile([P, 3], F32, tag="TBpq")
                nc.gpsimd.tensor_tensor(out=TBpq, in0=PQ[:, V:NS],
                                        in1=R[:, V:NS], op=AL.mult)
                TBdfp = sc.tile([P, 1], F32, tag="TBdfp")
                nc.gpsimd.tensor_tensor(out=TBdfp, in0=TBpq[:, 1:2],
                                        in1=TBpq[:, 2:3], op=AL.subtract)
                tq1 = sc.tile([P, CO], F32, tag="tq1")
                nc.gpsimd.tensor_tensor(out=tq1, in0=PQ[:, CO:V],
                                        in1=R[:, CO:V], op=AL.mult)
                tq2 = sc.tile([P, CO], F32, tag="tq2")
                nc.gpsimd.tensor_tensor(out=tq2, in0=PQ[:, 0:CO],
                                        in1=R[:, 0:CO], op=AL.mult)
                q1p = sc.tile([P, CO], F32, tag="q1p")
                nc.gpsimd.tensor_tensor(out=q1p, in0=tq1, in1=tq2,
                                        op=AL.subtract)
                q2p = sc.tile([P, CO], F32, tag="q2p")
                nc.vector.scalar_tensor_tensor(
                    out=q2p, in0=F8, scalar=TBdfp, in1=q1p,
                    op0=AL.mult, op1=AL.add)
                accP = sc.tile([P, 2], F32, tag="accP")
                yPQ = sc.tile([P, CO], F32, tag="yPQ")
                nc.vector.scalar_tensor_tensor(
                    out=yPQ, in0=q2p, scalar=TBpq[:, 0:1], in1=DI,
                    op0=AL.add, op1=AL.mult, accum_out=accP[:, 0:1])
                yPQf = sc.tile([P, CO], F32, tag="yPQf")
                nc.vector.scalar_tensor_tensor(
                    out=yPQf, in0=yPQ, scalar=1.0, in1=F8,
                    op0=AL.bypass, op1=AL.mult, accum_out=accP[:, 1:2])

                # --- [G] albe_B1 packed 2x2 (needs VBcp = -[S1B,S2B]) ---
                VBcp = sc.tile([P, 2], F32, tag="VBcp")
                nc.scalar.mul(VBcp, VB, -1.0)      # [A]
                S1m2Bp = sc.tile([P, 2], F32, tag="S1m2Bp")
                nc.gpsimd.tensor_tensor(out=S1m2Bp[:, 0:1],
                                        in0=VBcp[:, 0:1],
                                        in1=VBcp[:, 1:2], op=AL.subtract)
                nc.gpsimd.tensor_tensor(out=S1m2Bp[:, 1:2],
                                        in0=VBcp[:, 1:2],
                                        in1=VBcp[:, 0:1], op=AL.subtract)
                q2Bp = sc.tile([P, 2], F32, tag="q2Bp")
                nc.gpsimd.tensor_tensor(out=q2Bp[:, 0:1],
                                        in0=S1m2Bp[:, 0:1],
                                        in1=VUSS[:, 0:1], op=AL.mult)
                nc.gpsimd.tensor_tensor(out=q2Bp[:, 1:2],
                                        in0=S1m2Bp[:, 1:2],
                                        in1=VUSS[:, 0:1], op=AL.mult)
                ABBk = sc.tile([P, 2], F32, tag="ABBk")
                nc.gpsimd.tensor_tensor(out=ABBk, in0=AIBI[:, 1:3],
                                        in1=VBcp, op=AL.mult)
                ABB = sc.tile([P, 2], F32, tag="ABB")
                nc.gpsimd.tensor_tensor(out=ABB, in0=ABBk, in1=q2Bp,
                                        op=AL.add)

                # --- [V] alpha_aff readout + bcast ---
                qm_a = sc.tile([1, 1], F32, tag="qm_a")
                nc.vector.reduce_max(qm_a, qrow_a, axis=AX)
                qc_a = sc.tile([1, 1], F32, tag="qc_a")
                nc.vector.tensor_scalar(out=qc_a, in0=qm_a, scalar1=1.0,
                                        scalar2=None, op0=AL.max)
                qr_a = sc.tile([1, 1], F32, tag="qr_a")
                nc.vector.reciprocal(qr_a, qc_a)

                # --- [PE] MM4 (c-coeffs), B-a, MM5 (B(pq) sums) ---
                CC = ps.tile([P, 2], F32, tag="pscr")
                nc.tensor.matmul(CC, ONES, accC)
                AAFF = ps1.tile([P, 1], F32, tag="albc")
                nc.tensor.matmul(AAFF, ONES[0:1, :], qr_a)
                SPQ = ps.tile([P, 2], F32, tag="pscr")
                nc.tensor.matmul(SPQ, ONES, accP)

                # --- [V] albe_B1 tail + dxB1 ---
                albe_B1n = sc.tile([P, 2], F32, tag="albe_B1n")
                nc.vector.tensor_scalar(out=albe_B1n, in0=ABB,
                                        scalar1=DETI, scalar2=None,
                                        op0=AL.mult)
                dxq0 = sc.tile([P, CO], F32, tag="dxq0")
                nc.vector.scalar_tensor_tensor(
                    out=dxq0, in0=DI, scalar=albe_B1n[:, 0:1], in1=yB1,
                    op0=AL.mult, op1=AL.add)
                dxB1 = sc.tile([P, CO], F32, tag="dxB1")
                nc.vector.scalar_tensor_tensor(
                    out=dxB1, in0=DIF, scalar=albe_B1n[:, 1:2], in1=dxq0,
                    op0=AL.mult, op1=AL.add)
                c12 = sc.tile([P, 2], F32, tag="c12")
                nc.vector.tensor_tensor(out=c12, in0=CC[:, 0:2],
                                        in1=c12s, op=AL.add)

                # --- [A] SPQ negated copy; [V] albe_pq ---
                SPQs = sc.tile([P, 2], F32, tag="SPQs")
                nc.scalar.mul(SPQs, SPQ, -1.0)
                S1m2p = sc.tile([P, 1], F32, tag="S1m2p")
                nc.vector.tensor_tensor(out=S1m2p, in0=SPQs[:, 0:1],
                                        in1=SPQs[:, 1:2], op=AL.subtract)
                q2pp = sc.tile([P, 1], F32, tag="q2pp")
                nc.vector.tensor_tensor(out=q2pp, in0=S1m2p,
                                        in1=VUSS[:, 0:1], op=AL.mult)
                ABp = sc.tile([P, 2], F32, tag="ABp")
                nc.vector.tensor_scalar(out=ABp[:, 0:1], in0=AIBI[:, 1:2],
                                        scalar1=SPQs[:, 0:1], scalar2=q2pp,
                                        op0=AL.mult, op1=AL.add)
                nc.vector.tensor_scalar(out=ABp[:, 1:2], in0=ApSd,
                                        scalar1=SPQs[:, 1:2], scalar2=q2pp,
                                        op0=AL.mult, op1=AL.subtract)
                albe_pqn = sc.tile([P, 2], F32, tag="albe_pqn")
                nc.vector.tensor_scalar(out=albe_pqn, in0=ABp,
                                        scalar1=DETI, scalar2=None,
                                        op0=AL.mult)
                dxq1 = sc.tile([P, CO], F32, tag="dxq1")
                nc.vector.scalar_tensor_tensor(
                    out=dxq1, in0=DI, scalar=albe_pqn[:, 0:1], in1=yPQ,
                    op0=AL.mult, op1=AL.add)
                dxPQ = sc.tile([P, CO], F32, tag="dxPQ")
                nc.vector.scalar_tensor_tensor(
                    out=dxPQ, in0=DIF, scalar=albe_pqn[:, 1:2], in1=dxq1,
                    op0=AL.mult, op1=AL.add)
                dx0 = sc.tile([P, CO], F32, tag="dx0")
                nc.vector.tensor_tensor(out=dx0, in0=DXa, in1=dxPQ,
                                        op=AL.add)
                albe0n = sc.tile([P, 2], F32, tag="albe0n")
                nc.vector.tensor_tensor(out=albe0n, in0=albe_an,
                                        in1=albe_pqn, op=AL.add)
                rc0 = sc.tile([P, NS], F32, tag="rc0")
                nc.vector.tensor_tensor(out=rc0, in0=SZA, in1=PQ,
                                        op=AL.add)

                # --- [V] nsmu chain (needs AAFF) ---
                u1 = sc.tile([P, 1], F32, tag="u1")
                nc.vector.tensor_scalar(out=u1, in0=c12[:, 1:2],
                                        scalar1=AAFF, scalar2=c12[:, 0:1],
                                        op0=AL.mult, op1=AL.add)
                MAm = sc.tile([P, 1], F32, tag="MAm")
                nc.vector.tensor_scalar(out=MAm, in0=u1, scalar1=AAFF,
                                        scalar2=MUm, op0=AL.mult,
                                        op1=AL.add)
                MA3 = sc.tile([P, 1], F32, tag="MA3")
                nc.vector.tensor_scalar(out=MA3, in0=MAm, scalar1=MAm,
                                        scalar2=MAm, op0=AL.mult,
                                        op1=AL.mult)
                NSMU = sc.tile([P, 1], F32, tag="NSMU")
                nc.vector.tensor_tensor(out=NSMU, in0=MA3, in1=K,
                                        op=AL.mult)

                # --- [V] corrector assembly ---
                dxc = sc.tile([P, CO], F32, tag="dxc")
                nc.vector.scalar_tensor_tensor(
                    out=dxc, in0=dxB1, scalar=NSMU, in1=dx0,
                    op0=AL.mult, op1=AL.add)
                albe_cn = sc.tile([P, 2], F32, tag="albe_cn")
                nc.vector.scalar_tensor_tensor(
                    out=albe_cn, in0=albe_B1n, scalar=NSMU, in1=albe0n,
                    op0=AL.mult, op1=AL.add)
                rszc = sc.tile([P, NS], F32, tag="rszc")
                nc.vector.tensor_scalar(out=rszc, in0=rc0, scalar1=NSMU,
                                        scalar2=None, op0=AL.add)
                DSZc = sc.tile([P, 2 * NS], F32, tag="DSZc")
                nc.vector.tensor_scalar(out=DSZc[:, 0:CO], in0=dxc,
                                        scalar1=NPHI2, scalar2=None,
                                        op0=AL.add)
                nc.vector.tensor_scalar(out=DSZc[:, CO:V], in0=dxc,
                                        scalar1=-1.0, scalar2=NPHI,
                                        op0=AL.mult, op1=AL.add)
                SFPc = sc.tile([P, 2], F32, tag="SFPc")
                nc.gpsimd.tensor_tensor(out=SFPc, in0=AIBI[:, 0:2],
                                        in1=albe_cn, op=AL.mult)
                nc.gpsimd.tensor_tensor(out=DSZc[:, V:V + 2], in0=SFPc,
                                        in1=RPs[:, 0:2], op=AL.subtract)
                nc.gpsimd.tensor_tensor(out=DSZc[:, V + 2:NS], in0=NRPs2,
                                        in1=SFPc[:, 1:2], op=AL.subtract)
                uvc = sc.tile([P, NS], F32, tag="uvc")
                nc.vector.tensor_tensor(out=uvc, in0=SZ[:, NS:2 * NS],
                                        in1=DSZc[:, 0:NS], op=AL.mult)
                vvc = sc.tile([P, NS], F32, tag="vvc")
                nc.vector.tensor_tensor(out=vvc, in0=uvc, in1=rszc,
                                        op=AL.add)
                nc.vector.scalar_tensor_tensor(
                    out=DSZc[:, NS:2 * NS], in0=vvc, scalar=-1.0,
                    in1=R[:, 0:NS], op0=AL.mult, op1=AL.mult)
                Qc = sc.tile([P, 2 * NS], F32, tag="Qc")
                nc.vector.scalar_tensor_tensor(
                    out=Qc, in0=DSZc, scalar=-1.0, in1=R,
                    op0=AL.mult, op1=AL.mult)
                qpc = sc.tile([P, 1], F32, tag="qpc")
                nc.vector.reduce_max(qpc, Qc, axis=AX)

                # --- [PE] T-c, [V] readout, [PE] B-c ---
                qrow_c = psq.tile([1, P], F32, tag="qrow")
                nc.tensor.transpose(qrow_c, qpc, IDENT)
                qm_c = sc.tile([1, 1], F32, tag="qm_c")
                nc.vector.reduce_max(qm_c, qrow_c, axis=AX)
                qc_c = sc.tile([1, 1], F32, tag="qc_c")
                nc.vector.tensor_scalar(out=qc_c, in0=qm_c, scalar1=1.0,
                                        scalar2=1.0 / 0.99, op0=AL.max,
                                        op1=AL.mult)
                acr = sc.tile([1, 1], F32, tag="acr")
                nc.vector.reciprocal(acr, qc_c)
                ALC = ps1.tile([P, 1], F32, tag="albc")
                nc.tensor.matmul(ALC, ONES[0:1, :], acr)

                # --- [V] state updates (0.99 pre-folded into ALC) ---
                OneM = sc.tile([P, 1], F32, tag="OneM")
                nc.vector.tensor_scalar(out=OneM, in0=ALC, scalar1=-1.0,
                                        scalar2=1.0, op0=AL.mult,
                                        op1=AL.add)
                nc.vector.scalar_tensor_tensor(
                    out=XT, in0=dxc, scalar=ALC, in1=XT,
                    op0=AL.mult, op1=AL.add)
                nc.vector.scalar_tensor_tensor(
                    out=SZ, in0=DSZc, scalar=ALC, in1=SZ,
                    op0=AL.mult, op1=AL.add)
                nc.vector.tensor_scalar(out=SZ, in0=SZ, scalar1=CLAMP,
                                        scalar2=None, op0=AL.max)

                # --- [V] phi-state update (all 8 scaled cols at once) ---
                nc.vector.tensor_scalar(out=PH, in0=PH, scalar1=OneM,
                                        scalar2=None, op0=AL.mult)

            # ---------------- end projection ----------------
            accF = sc.tile([P, 2], F32, tag="accF")
            fxv = sc.tile([P, CO], F32, tag="fxv")
            nc.vector.scalar_tensor_tensor(
                out=fxv, in0=XT, scalar=1.0, in1=F8,
                op0=AL.bypass, op1=AL.mult, accum_out=accF[:, 1:2])
            nc.vector.reduce_sum(accF[:, 0:1], XT, axis=AX)
            SXF = ps.tile([P, 2], F32, tag="pscr")   # [Sx|Fx]
            nc.tensor.matmul(SXF, ONES, accF)

            R2 = sc.tile([P, 2 * NS], F32, tag="R")
            nc.vector.reciprocal(R2, SZ)
            W2 = sc.tile([P, NS], F32, tag="W")
            nc.vector.tensor_tensor(out=W2, in0=SZ[:, NS:2 * NS],
                                    in1=R2[:, 0:NS], op=AL.mult)
            D2 = sc.tile([P, CO], F32, tag="Dt")
            nc.vector.scalar_tensor_tensor(
                out=D2, in0=W2[:, 0:CO], scalar=EPS, in1=W2[:, CO:V],
                op0=AL.add, op1=AL.add)
            DI2 = sc.tile([P, CO], F32, tag="DI")
            nc.vector.reciprocal(DI2, D2)
            nc.vector.tensor_scalar(out=DI2, in0=DI2, scalar1=1e-4,
                                    scalar2=None, op0=AL.max)
            acc2f = sc.tile([P, 2], F32, tag="accA")
            DIF2 = sc.tile([P, CO], F32, tag="DIF")
            nc.vector.scalar_tensor_tensor(
                out=DIF2, in0=DI2, scalar=1.0, in1=F8,
                op0=AL.bypass, op1=AL.mult, accum_out=acc2f[:, 0:1])
            nc.vector.reduce_sum(acc2f[:, 1:2], DI2, axis=AX)
            VUS2p = ps.tile([P, 2], F32, tag="pscr")  # [Sv|Su]
            nc.tensor.matmul(VUS2p, ONES, acc2f)
            VUS2 = sc.tile([P, 2], F32, tag="VUSS")
            nc.scalar.copy(VUS2, VUS2p)

            GT3 = sc.tile([P, 3], F32, tag="GT3")     # actives [g0 gf1 gf2]
            nc.vector.tensor_tensor(out=GT3, in0=z_s, in1=s_s, op=AL.is_gt)
            d0 = sc.tile([P, 1], F32, tag="d0")
            nc.vector.tensor_scalar(out=d0, in0=s_s[:, 0:1],
                                    scalar1=SXF[:, 0:1], scalar2=-C_CAP,
                                    op0=AL.add, op1=AL.add)
            dfa = sc.tile([P, 1], F32, tag="dfa")
            nc.vector.tensor_scalar(out=dfa, in0=s_s[:, 1:2],
                                    scalar1=SXF[:, 1:2], scalar2=HFA,
                                    op0=AL.add, op1=AL.subtract)
            dfb = sc.tile([P, 1], F32, tag="dfb")
            nc.vector.tensor_scalar(out=dfb, in0=s_s[:, 2:3],
                                    scalar1=SXF[:, 1:2], scalar2=HFB,
                                    op0=AL.subtract, op1=AL.add)
            ub = sc.tile([P, 1], F32, tag="ub")
            nc.vector.tensor_tensor(out=ub, in0=GT3[:, 2:3], in1=dfb,
                                    op=AL.mult)
            df = sc.tile([P, 1], F32, tag="df")
            nc.vector.scalar_tensor_tensor(
                out=df, in0=GT3[:, 1:2], scalar=dfa, in1=ub,
                op0=AL.mult, op1=AL.subtract)
            gf = sc.tile([P, 1], F32, tag="gf")
            nc.vector.tensor_tensor(out=gf, in0=GT3[:, 1:2],
                                    in1=GT3[:, 2:3], op=AL.max)
            nnum0 = sc.tile([P, 1], F32, tag="nnum0")
            nc.vector.scalar_tensor_tensor(
                out=nnum0, in0=gf, scalar=df, in1=d0,
                op0=AL.mult, op1=AL.subtract)
            nden0 = sc.tile([P, 1], F32, tag="nden0")
            nc.vector.scalar_tensor_tensor(
                out=nden0, in0=gf, scalar=VUS2[:, 0:1], in1=VUS2[:, 1:2],
                op0=AL.mult, op1=AL.subtract)
            ddt = sc.tile([P, 1], F32, tag="ddt")
            nc.vector.tensor_scalar(out=ddt, in0=nden0, scalar1=nden0,
                                    scalar2=TINY, op0=AL.mult, op1=AL.add)
            rdd = sc.tile([P, 1], F32, tag="rdd")
            nc.vector.reciprocal(rdd, ddt)
            v0a = sc.tile([P, 1], F32, tag="v0a")
            nc.vector.tensor_tensor(out=v0a, in0=nnum0, in1=nden0,
                                    op=AL.mult)
            v0b = sc.tile([P, 1], F32, tag="v0b")
            nc.vector.tensor_tensor(out=v0b, in0=v0a, in1=rdd, op=AL.mult)
            v0 = sc.tile([P, 1], F32, tag="v0")
            nc.vector.tensor_tensor(out=v0, in0=GT3[:, 0:1], in1=v0b,
                                    op=AL.mult)
            sv2t = sc.tile([P, 1], F32, tag="sv2t")
            nc.vector.tensor_scalar(out=sv2t, in0=VUS2[:, 0:1],
                                    scalar1=VUS2[:, 0:1], scalar2=TINY,
                                    op0=AL.mult, op1=AL.add)
            rsv = sc.tile([P, 1], F32, tag="rsv")
            nc.vector.reciprocal(rsv, sv2t)
            u1f = sc.tile([P, 1], F32, tag="u1f")
            nc.vector.tensor_tensor(out=u1f, in0=df, in1=VUS2[:, 0:1],
                                    op=AL.mult)
            v1a = sc.tile([P, 1], F32, tag="v1a")
            nc.vector.tensor_tensor(out=v1a, in0=u1f, in1=rsv, op=AL.mult)
            w1 = sc.tile([P, 1], F32, tag="w1")
            nc.vector.tensor_tensor(out=w1, in0=gf, in1=v1a, op=AL.mult)
            omgf = sc.tile([P, 1], F32, tag="omgf")
            nc.vector.tensor_scalar(out=omgf, in0=gf, scalar1=-1.0,
                                    scalar2=1.0, op0=AL.mult, op1=AL.add)
            w3 = sc.tile([P, 1], F32, tag="w3")
            nc.vector.tensor_tensor(out=w3, in0=omgf, in1=v0, op=AL.mult)
            v1 = sc.tile([P, 1], F32, tag="v1")
            nc.vector.tensor_tensor(out=v1, in0=w1, in1=w3, op=AL.add)
            bee = sc.tile([P, 1], F32, tag="bee")
            nc.vector.tensor_tensor(out=bee, in0=v1, in1=v0,
                                    op=AL.subtract)
            corr = sc.tile([P, CO], F32, tag="corr")
            nc.vector.tensor_scalar(out=corr, in0=F8, scalar1=bee,
                                    scalar2=v0, op0=AL.mult, op1=AL.add)
            mcor = sc.tile([P, CO], F32, tag="mcor")
            nc.vector.tensor_tensor(out=mcor, in0=DI2, in1=corr,
                                    op=AL.mult)
            nc.vector.tensor_tensor(out=XT, in0=XT, in1=mcor,
                                    op=AL.subtract)
            nc.vector.tensor_scalar(out=XT, in0=XT, scalar1=0.0,
                                    scalar2=1.0, op0=AL.max, op1=AL.min)

            nc.sync.dma_start(out=o_ap, in_=XT)

    return nc


_CACHE: dict = {}


def _get_nc():
    if "nc" not in _CACHE:
        nc = bacc.Bacc(None, target_bir_lowering=False)
        _build(nc)
        nc.finalize()
        _CACHE["nc"] = nc
    return _CACHE["nc"]


def kernel(x: np.ndarray, indices_male: np.ndarray) -> np.ndarray:
    nc = _get_nc()
    base = {
        "x": np.ascontiguousarray(x, dtype=np.float32),
        "ind": np.ascontiguousarray(indices_male, dtype=np.int32),
        "ones": np.ones((P, P), dtype=np.float32),
        "ident": np.eye(P, dtype=np.float32),
    }
    in_maps = [dict(base) for _ in range(8)]
    res = run_bass_kernel_spmd(nc, in_maps, core_ids=list(range(8)))
    return np.asarray(res.results[0]["out"], dtype=np.float32)


if __name__ == "__main__":
    rng = np.random.default_rng(0)
    x = rng.standard_normal((1, N)).astype(np.float32)
    f = (np.arange(N) % 2).astype(np.int32)
    out = kernel(x, f)
    print("out", out.shape, out.dtype, out[0, :6], out.sum())


# revision 32
# speedup vs baseline: 1.1897x; 1.1897x over previous
"""Trainium2 Bass kernel for nn_CapLayerLP: box+cap+fairness QP via
primal-dual predictor-corrector interior point.

v2 vs the original baseline (359us):
- Warm start s0=z0=2.0 (residual scheme generalized to rp_p0 != 0):
  converges in 8 iterations instead of 16 (validated in fp32 sim with
  noise-perturbation robustness checks; rel err ~2e-4 vs 2e-2 gate).
- 3-direction decomposition: the Newton direction map is affine in the
  complementarity residual rsz, so the corrector is assembled as
  d_c = d_affine + B(ds_a*dz_a) + nsmu*B(1), where B (linear part, no
  rp/rx terms) is computed OFF the critical path: B(1) during stage A,
  B(pq) during the affine step-length PE round trips.
- mu_aff via quadratic expansion mu_aff*m = mu*m + a*c1 + a^2*c2
  (coefficients reduced before alpha is known -> no post-alpha
  reduction round trip).
- Uniform DSZ=[ds|dz] layout -> one fused ratio tile for the step
  length and one fused s/z update.
- Narrow scalar algebra spread onto GpSimd + Activation engines so the
  DVE critical path stays on the wide tiles.

Sharding: batch is 1 and the solve is latency-bound; replicated on all
8 cores, core 0's output returned.
"""
import os

import numpy as np

import concourse.bass as bass
import concourse.bacc as bacc
import concourse.tile as tile
from concourse import mybir
from concourse.bass_utils import run_bass_kernel_spmd

AL = mybir.AluOpType
F32 = mybir.dt.float32
AX = mybir.AxisListType.X

N = 1024
P = 128
CO = N // P            # 8
V = 2 * CO             # 16
NS = V + 3             # 19
C_CAP = 10.0
EPS = 1e-4
INIT = 2.0             # warm-start scale for s and z
ITERS = int(os.environ.get("KD_ITERS", "8"))
M_CONST = 2 * N + 3
CLAMP = 1e-30
TINY = 1e-12


def _build(nc: bass.Bass):
    x_d = nc.dram_tensor("x", [1, N], F32, kind="ExternalInput")
    f_d = nc.dram_tensor("ind", [N], mybir.dt.int32, kind="ExternalInput")
    ones_d = nc.dram_tensor("ones", [P, P], F32, kind="ExternalInput")
    ident_d = nc.dram_tensor("ident", [P, P], F32, kind="ExternalInput")
    out_d = nc.dram_tensor("out", [1, N], F32, kind="ExternalOutput")

    x_ap = x_d[:, :].rearrange("a (p c) -> a p c", p=P)[0]
    f_ap = f_d[:].rearrange("(p c) -> p c", p=P)
    o_ap = out_d[:, :].rearrange("a (p c) -> a p c", p=P)[0]

    with tile.TileContext(nc) as tc:
        with (
            tc.tile_pool(name="const", bufs=1) as cns,
            tc.tile_pool(name="state", bufs=1) as st,
            tc.tile_pool(name="scr", bufs=2) as sc,
            tc.tile_pool(name="psum", bufs=2, space="PSUM") as ps,
            tc.tile_pool(name="psum1", bufs=2, space="PSUM") as ps1,
            tc.tile_pool(name="psumq", bufs=2, space="PSUM") as psq,
        ):
            # ---------------- setup ----------------
            ONES = cns.tile([P, P], F32)
            IDENT = cns.tile([P, P], F32)
            nc.sync.dma_start(out=ONES[:, :], in_=ones_d[:, :])
            nc.sync.dma_start(out=IDENT[:, :], in_=ident_d[:, :])

            F8 = cns.tile([P, CO], F32)
            nc.gpsimd.dma_start(out=F8, in_=f_ap)  # int32 -> f32 cast

            XIN = cns.tile([P, CO], F32)
            nc.sync.dma_start(out=XIN, in_=x_ap)
            RX0 = cns.tile([P, CO], F32)           # rx0 = INIT - x_in
            nc.vector.tensor_scalar(out=RX0, in0=XIN, scalar1=-1.0,
                                    scalar2=INIT, op0=AL.mult, op1=AL.add)

            XT = st.tile([P, CO], F32)
            nc.vector.memset(XT, 0.0)
            SZ = st.tile([P, 2 * NS], F32)
            nc.vector.memset(SZ, INIT)
            # PH = [phi | 2phi | -phi | -2phi]
            PH = st.tile([P, 4], F32)
            nc.vector.memset(PH[:, 0:1], 1.0)
            nc.vector.memset(PH[:, 1:2], 2.0)
            nc.vector.memset(PH[:, 2:3], -1.0)
            nc.vector.memset(PH[:, 3:4], -2.0)
            PHI = PH[:, 0:1]
            PH2 = PH[:, 1:2]
            NPHI = PH[:, 2:3]
            NPHI2 = PH[:, 3:4]

            facc = sc.tile([P, 1], F32, tag="facc")
            nc.vector.reduce_sum(facc, F8, axis=AX)
            NMp = ps.tile([P, 1], F32, tag="pscr")
            nc.tensor.matmul(NMp, ONES, facc)      # Nm replicated

            # RF = rp0 scalars [INIT-C, INIT-C*Nm/N-1, INIT+C*Nm/N]
            RF = cns.tile([P, 3], F32)
            nc.vector.memset(RF[:, 0:1], INIT - C_CAP)
            nc.vector.tensor_scalar(out=RF[:, 1:2], in0=NMp,
                                    scalar1=-C_CAP / N, scalar2=INIT - 1.0,
                                    op0=AL.mult, op1=AL.add)
            nc.vector.tensor_scalar(out=RF[:, 2:3], in0=NMp,
                                    scalar1=C_CAP / N, scalar2=INIT,
                                    op0=AL.mult, op1=AL.add)
            NRF2 = cns.tile([P, 1], F32)           # -RF[2]
            nc.vector.tensor_scalar(out=NRF2, in0=NMp, scalar1=-C_CAP / N,
                                    scalar2=-INIT, op0=AL.mult, op1=AL.add)
            HFA = cns.tile([P, 1], F32)            # C*Nm/N + 1 (= hf1)
            nc.vector.tensor_scalar(out=HFA, in0=NMp, scalar1=C_CAP / N,
                                    scalar2=1.0, op0=AL.mult, op1=AL.add)
            HFB = cns.tile([P, 1], F32)            # C*Nm/N (= -hf2)
            nc.vector.tensor_scalar(out=HFB, in0=NMp, scalar1=C_CAP / N,
                                    scalar2=None, op0=AL.mult)

            AIBI = st.tile([P, 3], F32)            # [AINV | BINV | ApSd]

            s_v = SZ[:, 0:V]
            s_s = SZ[:, V:NS]
            z_v = SZ[:, NS:NS + V]
            z_s = SZ[:, NS + V:2 * NS]

            # ---------------- iterations ----------------
            for it in range(ITERS):
                RPs = sc.tile([P, 3], F32, tag="RPs")
                nc.vector.tensor_scalar(out=RPs, in0=RF, scalar1=PHI,
                                        scalar2=None, op0=AL.mult)
                NRPs2 = sc.tile([P, 1], F32, tag="NRPs2")
                nc.vector.tensor_scalar(out=NRPs2, in0=NRF2, scalar1=PHI,
                                        scalar2=None, op0=AL.mult)
                # --- [V] stage A wide chain ---
                R = sc.tile([P, 2 * NS], F32, tag="R")
                nc.vector.reciprocal(R, SZ)
                W = sc.tile([P, NS], F32, tag="W")
                nc.vector.tensor_tensor(out=W, in0=SZ[:, NS:2 * NS],
                                        in1=R[:, 0:NS], op=AL.mult)
                Dt = sc.tile([P, CO], F32, tag="Dt")
                nc.vector.scalar_tensor_tensor(
                    out=Dt, in0=W[:, 0:CO], scalar=EPS, in1=W[:, CO:V],
                    op0=AL.add, op1=AL.add)
                DI = sc.tile([P, CO], F32, tag="DI")
                nc.vector.reciprocal(DI, Dt)
                accA = sc.tile([P, 3], F32, tag="accA")  # [Sv|Su|mac]
                DIF = sc.tile([P, CO], F32, tag="DIF")
                nc.vector.scalar_tensor_tensor(
                    out=DIF, in0=DI, scalar=1.0, in1=F8,
                    op0=AL.bypass, op1=AL.mult, accum_out=accA[:, 0:1])
                nc.vector.reduce_sum(accA[:, 1:2], DI, axis=AX)
                SZA = sc.tile([P, NS], F32, tag="SZA")   # rsz_a = s*z
                nc.vector.scalar_tensor_tensor(
                    out=SZA[:, 0:V], in0=s_v, scalar=1.0, in1=z_v,
                    op0=AL.bypass, op1=AL.mult, accum_out=accA[:, 2:3])
                # --- [G] trio chains (SZA trio, NT, B(1) parts, Bt) ---
                nc.gpsimd.tensor_tensor(out=AIBI[:, 0:1],
                                        in0=SZ[:, V:V + 1],
                                        in1=R[:, NS + V:NS + V + 1],
                                        op=AL.mult)
                nc.gpsimd.tensor_tensor(out=SZA[:, V:NS], in0=s_s, in1=z_s,
                                        op=AL.mult)
                u_nt = sc.tile([P, 3], F32, tag="u_nt")
                nc.gpsimd.tensor_tensor(out=u_nt, in0=z_s, in1=RPs,
                                        op=AL.mult)
                v_nt = sc.tile([P, 3], F32, tag="v_nt")
                nc.gpsimd.tensor_tensor(out=v_nt, in0=SZA[:, V:NS],
                                        in1=u_nt, op=AL.subtract)
                NTa = sc.tile([P, 3], F32, tag="NTa")
                nc.gpsimd.tensor_tensor(out=NTa, in0=v_nt, in1=R[:, V:NS],
                                        op=AL.mult)
                NTDF = sc.tile([P, 1], F32, tag="NTDF")
                nc.gpsimd.tensor_tensor(out=NTDF, in0=NTa[:, 1:2],
                                        in1=NTa[:, 2:3], op=AL.subtract)
                Bt = sc.tile([P, 1], F32, tag="Bt")
                nc.gpsimd.tensor_tensor(out=Bt, in0=W[:, V + 1:V + 2],
                                        in1=W[:, V + 2:NS], op=AL.add)
                # --- [PE] MM1: stage-A sums ---
                VUS = ps.tile([P, 3], F32, tag="pscr")
                nc.tensor.matmul(VUS, ONES, accA)
                VUSS = sc.tile([P, 3], F32, tag="VUSS")
                nc.scalar.copy(VUSS, VUS)

                # --- [V] affine rhs chain ---
                tmr = sc.tile([P, CO], F32, tag="tmr")
                nc.vector.scalar_tensor_tensor(
                    out=tmr, in0=SZ[:, NS:NS + CO], scalar=PH2,
                    in1=SZA[:, 0:CO], op0=AL.mult, op1=AL.subtract)
                tm = sc.tile([P, CO], F32, tag="tm")
                nc.vector.tensor_tensor(out=tm, in0=tmr, in1=R[:, 0:CO],
                                        op=AL.mult)
                A1 = sc.tile([P, CO], F32, tag="A1")
                nc.vector.scalar_tensor_tensor(
                    out=A1, in0=RX0, scalar=NPHI, in1=tm,
                    op0=AL.mult, op1=AL.add)
                tppr = sc.tile([P, CO], F32, tag="tppr")
                nc.vector.scalar_tensor_tensor(
                    out=tppr, in0=SZ[:, NS + CO:NS + V], scalar=PHI,
                    in1=SZA[:, CO:V], op0=AL.mult, op1=AL.subtract)
                tppn = sc.tile([P, CO], F32, tag="tppn")
                nc.vector.tensor_tensor(out=tppn, in0=tppr,
                                        in1=R[:, CO:V], op=AL.mult)
                A2 = sc.tile([P, CO], F32, tag="A2")
                nc.vector.tensor_tensor(out=A2, in0=A1, in1=tppn,
                                        op=AL.subtract)
                B1a = sc.tile([P, CO], F32, tag="B1a")
                nc.vector.scalar_tensor_tensor(
                    out=B1a, in0=F8, scalar=NTDF, in1=A2,
                    op0=AL.mult, op1=AL.add)
                accS = sc.tile([P, 2], F32, tag="accS")
                Ya = sc.tile([P, CO], F32, tag="Ya")
                nc.vector.scalar_tensor_tensor(
                    out=Ya, in0=B1a, scalar=NTa[:, 0:1], in1=DI,
                    op0=AL.add, op1=AL.mult, accum_out=accS[:, 0:1])
                Yaf = sc.tile([P, CO], F32, tag="Yaf")
                nc.vector.scalar_tensor_tensor(
                    out=Yaf, in0=Ya, scalar=1.0, in1=F8,
                    op0=AL.bypass, op1=AL.mult, accum_out=accS[:, 1:2])
                # --- [PE] MM3 (S12a) ---
                S12a = ps.tile([P, 2], F32, tag="pscr")
                nc.tensor.matmul(S12a, ONES, accS)

                # --- [G] det chain (needs VUSS, BINV) ---
                Sd = sc.tile([P, 1], F32, tag="Sd")
                nc.gpsimd.tensor_tensor(out=Sd, in0=VUSS[:, 1:2],
                                        in1=VUSS[:, 0:1], op=AL.subtract)
                SpSd = sc.tile([P, 1], F32, tag="SpSd")
                nc.gpsimd.tensor_tensor(out=SpSd, in0=VUSS[:, 0:1],
                                        in1=Sd, op=AL.add)
                T3 = sc.tile([P, 1], F32, tag="T3")
                nc.gpsimd.tensor_tensor(out=T3, in0=VUSS[:, 0:1], in1=Sd,
                                        op=AL.mult)
                nc.gpsimd.tensor_tensor(out=AIBI[:, 2:3], in0=AIBI[:, 0:1],
                                        in1=Sd, op=AL.add)
                ApSd = AIBI[:, 2:3]
                M22t = sc.tile([P, 1], F32, tag="M22t")
                nc.gpsimd.tensor_tensor(out=M22t, in0=AIBI[:, 0:1],
                                        in1=VUSS[:, 0:1], op=AL.mult)

                # --- [V] BINV ---
                nc.vector.reciprocal(AIBI[:, 1:2], Bt)

                # --- [G] finish det: det = AINV*Sv + BINV*(AINV+SpSd*?) ---
                # det = AINV*(BINV + Sv) + BINV*(Sv + Sd) + Sv*Sd
                #     = AINV*BINV + AINV*Sv(=M22t) ... use:
                # T2 = BINV*(AINV + SpSd) = AINV*BINV + BINV*(Sv+Sd)
                ApSp = sc.tile([P, 1], F32, tag="ApSp")
                nc.gpsimd.tensor_tensor(out=ApSp, in0=AIBI[:, 0:1],
                                        in1=SpSd, op=AL.add)
                T2 = sc.tile([P, 1], F32, tag="T2")
                nc.gpsimd.tensor_tensor(out=T2, in0=AIBI[:, 1:2],
                                        in1=ApSp, op=AL.mult)
                qb = sc.tile([P, 1], F32, tag="qb")
                nc.gpsimd.tensor_tensor(out=qb, in0=T2, in1=T3, op=AL.add)
                DETt = sc.tile([P, 1], F32, tag="DETt")
                nc.gpsimd.tensor_tensor(out=DETt, in0=M22t, in1=qb,
                                        op=AL.add)
                DETI = sc.tile([P, 1], F32, tag="DETI")
                nc.vector.reciprocal(DETI, DETt)
                msca = sc.tile([P, 1], F32, tag="msca")
                nc.gpsimd.tensor_tensor(out=msca, in0=SZA[:, V:V + 1],
                                        in1=SZA[:, V + 1:V + 2], op=AL.add)
                mscb = sc.tile([P, 1], F32, tag="mscb")
                nc.gpsimd.tensor_tensor(out=mscb, in0=msca,
                                        in1=SZA[:, V + 2:NS], op=AL.add)
                # B(1): TB = R_s trio; q1 = Rp - Rm
                TBdf1 = sc.tile([P, 1], F32, tag="TBdf1")
                nc.gpsimd.tensor_tensor(out=TBdf1, in0=R[:, V + 1:V + 2],
                                        in1=R[:, V + 2:NS], op=AL.subtract)
                q1B = sc.tile([P, CO], F32, tag="q1B")
                nc.gpsimd.tensor_tensor(out=q1B, in0=R[:, CO:V],
                                        in1=R[:, 0:CO], op=AL.subtract)

                # --- [A] negated psum copies ---
                S12s = sc.tile([P, 2], F32, tag="S12s")
                nc.scalar.mul(S12s, S12a, -1.0)

                # --- [V] albe_a (negated) + dx_a ---
                S1m2a = sc.tile([P, 1], F32, tag="S1m2a")
                nc.vector.tensor_tensor(out=S1m2a, in0=S12s[:, 0:1],
                                        in1=S12s[:, 1:2], op=AL.subtract)
                q2a = sc.tile([P, 1], F32, tag="q2a")
                nc.vector.tensor_tensor(out=q2a, in0=S1m2a,
                                        in1=VUSS[:, 0:1], op=AL.mult)
                ABa = sc.tile([P, 2], F32, tag="ABa")
                nc.vector.tensor_scalar(out=ABa[:, 0:1], in0=AIBI[:, 1:2],
                                        scalar1=S12s[:, 0:1], scalar2=q2a,
                                        op0=AL.mult, op1=AL.add)
                nc.vector.tensor_scalar(out=ABa[:, 1:2], in0=ApSd,
                                        scalar1=S12s[:, 1:2], scalar2=q2a,
                                        op0=AL.mult, op1=AL.subtract)
                albe_an = sc.tile([P, 2], F32, tag="albe_an")
                nc.vector.tensor_scalar(out=albe_an, in0=ABa,
                                        scalar1=DETI, scalar2=None,
                                        op0=AL.mult)
                t1a = sc.tile([P, CO], F32, tag="t1a")
                nc.vector.scalar_tensor_tensor(
                    out=t1a, in0=DI, scalar=albe_an[:, 0:1], in1=Ya,
                    op0=AL.mult, op1=AL.add)
                DXa = sc.tile([P, CO], F32, tag="DXa")
                nc.vector.scalar_tensor_tensor(
                    out=DXa, in0=DIF, scalar=albe_an[:, 1:2], in1=t1a,
                    op0=AL.mult, op1=AL.add)

                # --- ds_a: [V] m/p blocks, [G] trio ---
                DSZa = sc.tile([P, 2 * NS], F32, tag="DSZa")
                nc.vector.tensor_scalar(out=DSZa[:, 0:CO], in0=DXa,
                                        scalar1=NPHI2, scalar2=None,
                                        op0=AL.add)
                nc.vector.tensor_scalar(out=DSZa[:, CO:V], in0=DXa,
                                        scalar1=-1.0, scalar2=NPHI,
                                        op0=AL.mult, op1=AL.add)
                SFPa = sc.tile([P, 2], F32, tag="SFPa")
                nc.gpsimd.tensor_tensor(out=SFPa, in0=AIBI[:, 0:2],
                                        in1=albe_an, op=AL.mult)
                nc.gpsimd.tensor_tensor(out=DSZa[:, V:V + 2],
                                        in0=SFPa, in1=RPs[:, 0:2],
                                        op=AL.subtract)
                nc.gpsimd.tensor_tensor(out=DSZa[:, V + 2:NS], in0=NRPs2,
                                        in1=SFPa[:, 1:2], op=AL.subtract)

                # --- [V] dz_a (19-wide incl trio) + steplen-a ---
                uva = sc.tile([P, NS], F32, tag="uva")
                nc.vector.tensor_tensor(out=uva, in0=SZ[:, NS:2 * NS],
                                        in1=DSZa[:, 0:NS], op=AL.mult)
                vva = sc.tile([P, NS], F32, tag="vva")
                nc.vector.tensor_tensor(out=vva, in0=uva, in1=SZA,
                                        op=AL.add)
                nc.vector.scalar_tensor_tensor(
                    out=DSZa[:, NS:2 * NS], in0=vva, scalar=-1.0,
                    in1=R[:, 0:NS], op0=AL.mult, op1=AL.mult)
                Qa = sc.tile([P, 2 * NS], F32, tag="Qa")
                nc.vector.scalar_tensor_tensor(
                    out=Qa, in0=DSZa, scalar=-1.0, in1=R,
                    op0=AL.mult, op1=AL.mult)
                qpa = sc.tile([P, 1], F32, tag="qpa")
                nc.vector.reduce_max(qpa, Qa, axis=AX)

                # --- [PE] T-a ---
                qrow_a = psq.tile([1, P], F32, tag="qrow")
                nc.tensor.transpose(qrow_a, qpa, IDENT)

                # --- [V] pq + mu_aff coefficients (alpha-independent) ---
                PQ = sc.tile([P, NS], F32, tag="PQ")
                accC = sc.tile([P, 2], F32, tag="accC")  # [c1v|c2v]
                nc.vector.scalar_tensor_tensor(
                    out=PQ[:, 0:V], in0=DSZa[:, 0:V], scalar=1.0,
                    in1=DSZa[:, NS:NS + V], op0=AL.bypass, op1=AL.mult,
                    accum_out=accC[:, 1:2])
                cz1 = sc.tile([P, V], F32, tag="cz1")
                nc.vector.tensor_tensor(out=cz1, in0=SZ[:, 0:V],
                                        in1=DSZa[:, NS:NS + V], op=AL.mult)
                czt = sc.tile([P, V], F32, tag="czt")
                nc.vector.scalar_tensor_tensor(
                    out=czt, in0=cz1, scalar=1.0, in1=uva[:, 0:V],
                    op0=AL.bypass, op1=AL.add, accum_out=accC[:, 0:1])
                # B(1) y-parts on V (T-a window), then MM2
                q2B = sc.tile([P, CO], F32, tag="q2B")
                nc.vector.scalar_tensor_tensor(
                    out=q2B, in0=F8, scalar=TBdf1, in1=q1B,
                    op0=AL.mult, op1=AL.add)
                accB = sc.tile([P, 2], F32, tag="accB")
                yB1 = sc.tile([P, CO], F32, tag="yB1")
                nc.vector.scalar_tensor_tensor(
                    out=yB1, in0=q2B, scalar=R[:, V:V + 1], in1=DI,
                    op0=AL.add, op1=AL.mult, accum_out=accB[:, 0:1])
                yB1f = sc.tile([P, CO], F32, tag="yB1f")
                nc.vector.scalar_tensor_tensor(
                    out=yB1f, in0=yB1, scalar=1.0, in1=F8,
                    op0=AL.bypass, op1=AL.mult, accum_out=accB[:, 1:2])
                VB = ps.tile([P, 2], F32, tag="pscr")
                nc.tensor.matmul(VB, ONES, accB)

                # --- [G] trio pq + cz sums ---
                nc.gpsimd.tensor_tensor(out=PQ[:, V:NS],
                                        in0=DSZa[:, V:NS],
                                        in1=DSZa[:, NS + V:2 * NS],
                                        op=AL.mult)
                czs1 = sc.tile([P, 3], F32, tag="czs1")
                nc.gpsimd.tensor_tensor(out=czs1, in0=s_s,
                                        in1=DSZa[:, NS + V:2 * NS],
                                        op=AL.mult)
                czs = sc.tile([P, 3], F32, tag="czs")
                nc.gpsimd.tensor_tensor(out=czs, in0=czs1,
                                        in1=uva[:, V:NS], op=AL.add)
                c1sa = sc.tile([P, 1], F32, tag="c1sa")
                nc.gpsimd.tensor_tensor(out=c1sa, in0=czs[:, 0:1],
                                        in1=czs[:, 1:2], op=AL.add)
                c2sa = sc.tile([P, 1], F32, tag="c2sa")
                nc.gpsimd.tensor_tensor(out=c2sa, in0=PQ[:, V:V + 1],
                                        in1=PQ[:, V + 1:V + 2], op=AL.add)
                c12s = sc.tile([P, 2], F32, tag="c12s")
                nc.gpsimd.tensor_tensor(out=c12s[:, 0:1], in0=c1sa,
                                        in1=czs[:, 2:3], op=AL.add)
                nc.gpsimd.tensor_tensor(out=c12s[:, 1:2], in0=c2sa,
                                        in1=PQ[:, V + 2:NS], op=AL.add)

                # --- [G] B(pq) prep (fills the T-a PE window) ---
                TBpq = sc.tile([P, 3], F32, tag="TBpq")
                nc.gpsimd.tensor_tensor(out=TBpq, in0=PQ[:, V:NS],
                                        in1=R[:, V:NS], op=AL.mult)
                TBdfp = sc.tile([P, 1], F32, tag="TBdfp")
                nc.gpsimd.tensor_tensor(out=TBdfp, in0=TBpq[:, 1:2],
                                        in1=TBpq[:, 2:3], op=AL.subtract)
                tq1 = sc.tile([P, CO], F32, tag="tq1")
                nc.gpsimd.tensor_tensor(out=tq1, in0=PQ[:, CO:V],
                                        in1=R[:, CO:V], op=AL.mult)
                tq2 = sc.tile([P, CO], F32, tag="tq2")
                nc.gpsimd.tensor_tensor(out=tq2, in0=PQ[:, 0:CO],
                                        in1=R[:, 0:CO], op=AL.mult)
                q1p = sc.tile([P, CO], F32, tag="q1p")
                nc.gpsimd.tensor_tensor(out=q1p, in0=tq1, in1=tq2,
                                        op=AL.subtract)
                q2p = sc.tile([P, CO], F32, tag="q2p")
                nc.vector.scalar_tensor_tensor(
                    out=q2p, in0=F8, scalar=TBdfp, in1=q1p,
                    op0=AL.mult, op1=AL.add)
                accP = sc.tile([P, 2], F32, tag="accP")
                yPQ = sc.tile([P, CO], F32, tag="yPQ")
                nc.vector.scalar_tensor_tensor(
                    out=yPQ, in0=q2p, scalar=TBpq[:, 0:1], in1=DI,
                    op0=AL.add, op1=AL.mult, accum_out=accP[:, 0:1])
                yPQf = sc.tile([P, CO], F32, tag="yPQf")
                nc.vector.scalar_tensor_tensor(
                    out=yPQf, in0=yPQ, scalar=1.0, in1=F8,
                    op0=AL.bypass, op1=AL.mult, accum_out=accP[:, 1:2])

                # --- [G] albe_B1 packed 2x2 (needs VBcp = -[S1B,S2B]) ---
                VBcp = sc.tile([P, 2], F32, tag="VBcp")
                nc.scalar.mul(VBcp, VB, -1.0)      # [A]
                S1m2Bp = sc.tile([P, 2], F32, tag="S1m2Bp")
                nc.gpsimd.tensor_tensor(out=S1m2Bp[:, 0:1],
                                        in0=VBcp[:, 0:1],
                                        in1=VBcp[:, 1:2], op=AL.subtract)
                nc.gpsimd.tensor_tensor(out=S1m2Bp[:, 1:2],
                                        in0=VBcp[:, 1:2],
                                        in1=VBcp[:, 0:1], op=AL.subtract)
                q2Bp = sc.tile([P, 2], F32, tag="q2Bp")
                nc.gpsimd.tensor_tensor(out=q2Bp[:, 0:1],
                                        in0=S1m2Bp[:, 0:1],
                                        in1=VUSS[:, 0:1], op=AL.mult)
                nc.gpsimd.tensor_tensor(out=q2Bp[:, 1:2],
                                        in0=S1m2Bp[:, 1:2],
                                        in1=VUSS[:, 0:1], op=AL.mult)
                ABBk = sc.tile([P, 2], F32, tag="ABBk")
                nc.gpsimd.tensor_tensor(out=ABBk, in0=AIBI[:, 1:3],
                                        in1=VBcp, op=AL.mult)
                ABB = sc.tile([P, 2], F32, tag="ABB")
                nc.gpsimd.tensor_tensor(out=ABB, in0=ABBk, in1=q2Bp,
                                        op=AL.add)

                # --- [V] alpha_aff readout + bcast ---
                qm_a = sc.tile([1, 1], F32, tag="qm_a")
                nc.vector.reduce_max(qm_a, qrow_a, axis=AX)
                qc_a = sc.tile([1, 1], F32, tag="qc_a")
                nc.vector.tensor_scalar(out=qc_a, in0=qm_a, scalar1=1.0,
                                        scalar2=None, op0=AL.max)
                qr_a = sc.tile([1, 1], F32, tag="qr_a")
                nc.vector.reciprocal(qr_a, qc_a)
                MUm = sc.tile([P, 1], F32, tag="MUm")
                nc.vector.tensor_tensor(out=MUm, in0=mscb,
                                        in1=VUSS[:, 2:3], op=AL.add)
                mui = sc.tile([P, 1], F32, tag="mui")
                nc.vector.reciprocal(mui, MUm)
                K = sc.tile([P, 1], F32, tag="K")
                nc.vector.tensor_scalar(out=K, in0=mui, scalar1=mui,
                                        scalar2=-1.0 / M_CONST,
                                        op0=AL.mult, op1=AL.mult)

                # --- [PE] MM4 (c-coeffs), B-a, MM5 (B(pq) sums) ---
                CC = ps.tile([P, 2], F32, tag="pscr")
                nc.tensor.matmul(CC, ONES, accC)
                AAFF = ps1.tile([P, 1], F32, tag="albc")
                nc.tensor.matmul(AAFF, ONES[0:1, :], qr_a)
                SPQ = ps.tile([P, 2], F32, tag="pscr")
                nc.tensor.matmul(SPQ, ONES, accP)

                # --- [V] albe_B1 tail + dxB1 ---
                albe_B1n = sc.tile([P, 2], F32, tag="albe_B1n")
                nc.vector.tensor_scalar(out=albe_B1n, in0=ABB,
                                        scalar1=DETI, scalar2=None,
                                        op0=AL.mult)
                dxq0 = sc.tile([P, CO], F32, tag="dxq0")
                nc.vector.scalar_tensor_tensor(
                    out=dxq0, in0=DI, scalar=albe_B1n[:, 0:1], in1=yB1,
                    op0=AL.mult, op1=AL.add)
                dxB1 = sc.tile([P, CO], F32, tag="dxB1")
                nc.vector.scalar_tensor_tensor(
                    out=dxB1, in0=DIF, scalar=albe_B1n[:, 1:2], in1=dxq0,
                    op0=AL.mult, op1=AL.add)
                c12 = sc.tile([P, 2], F32, tag="c12")
                nc.vector.tensor_tensor(out=c12, in0=CC[:, 0:2],
                                        in1=c12s, op=AL.add)

                # --- [A] SPQ negated copy; [V] albe_pq ---
                SPQs = sc.tile([P, 2], F32, tag="SPQs")
                nc.scalar.mul(SPQs, SPQ, -1.0)
                S1m2p = sc.tile([P, 1], F32, tag="S1m2p")
                nc.vector.tensor_tensor(out=S1m2p, in0=SPQs[:, 0:1],
                                        in1=SPQs[:, 1:2], op=AL.subtract)
                q2pp = sc.tile([P, 1], F32, tag="q2pp")
                nc.vector.tensor_tensor(out=q2pp, in0=S1m2p,
                                        in1=VUSS[:, 0:1], op=AL.mult)
                ABp = sc.tile([P, 2], F32, tag="ABp")
                nc.vector.tensor_scalar(out=ABp[:, 0:1], in0=AIBI[:, 1:2],
                                        scalar1=SPQs[:, 0:1], scalar2=q2pp,
                                        op0=AL.mult, op1=AL.add)
                nc.vector.tensor_scalar(out=ABp[:, 1:2], in0=ApSd,
                                        scalar1=SPQs[:, 1:2], scalar2=q2pp,
                                        op0=AL.mult, op1=AL.subtract)
                albe_pqn = sc.tile([P, 2], F32, tag="albe_pqn")
                nc.vector.tensor_scalar(out=albe_pqn, in0=ABp,
                                        scalar1=DETI, scalar2=None,
                                        op0=AL.mult)
                dxq1 = sc.tile([P, CO], F32, tag="dxq1")
                nc.vector.scalar_tensor_tensor(
                    out=dxq1, in0=DI, scalar=albe_pqn[:, 0:1], in1=yPQ,
                    op0=AL.mult, op1=AL.add)
                dxPQ = sc.tile([P, CO], F32, tag="dxPQ")
                nc.vector.scalar_tensor_tensor(
                    out=dxPQ, in0=DIF, scalar=albe_pqn[:, 1:2], in1=dxq1,
                    op0=AL.mult, op1=AL.add)
                dx0 = sc.tile([P, CO], F32, tag="dx0")
                nc.vector.tensor_tensor(out=dx0, in0=DXa, in1=dxPQ,
                                        op=AL.add)
                albe0n = sc.tile([P, 2], F32, tag="albe0n")
                nc.vector.tensor_tensor(out=albe0n, in0=albe_an,
                                        in1=albe_pqn, op=AL.add)
                rc0 = sc.tile([P, NS], F32, tag="rc0")
                nc.vector.tensor_tensor(out=rc0, in0=SZA, in1=PQ,
                                        op=AL.add)

                # --- [V] nsmu chain (needs AAFF) ---
                u1 = sc.tile([P, 1], F32, tag="u1")
                nc.vector.tensor_scalar(out=u1, in0=c12[:, 1:2],
                                        scalar1=AAFF, scalar2=c12[:, 0:1],
                                        op0=AL.mult, op1=AL.add)
                MAm = sc.tile([P, 1], F32, tag="MAm")
                nc.vector.tensor_scalar(out=MAm, in0=u1, scalar1=AAFF,
                                        scalar2=MUm, op0=AL.mult,
                                        op1=AL.add)
                MA3 = sc.tile([P, 1], F32, tag="MA3")
                nc.vector.tensor_scalar(out=MA3, in0=MAm, scalar1=MAm,
                                        scalar2=MAm, op0=AL.mult,
                                        op1=AL.mult)
                NSMU = sc.tile([P, 1], F32, tag="NSMU")
                nc.vector.tensor_tensor(out=NSMU, in0=MA3, in1=K,
                                        op=AL.mult)

                # --- [V] corrector assembly ---
                dxc = sc.tile([P, CO], F32, tag="dxc")
                nc.vector.scalar_tensor_tensor(
                    out=dxc, in0=dxB1, scalar=NSMU, in1=dx0,
                    op0=AL.mult, op1=AL.add)
                albe_cn = sc.tile([P, 2], F32, tag="albe_cn")
                nc.vector.scalar_tensor_tensor(
                    out=albe_cn, in0=albe_B1n, scalar=NSMU, in1=albe0n,
                    op0=AL.mult, op1=AL.add)
                rszc = sc.tile([P, NS], F32, tag="rszc")
                nc.vector.tensor_scalar(out=rszc, in0=rc0, scalar1=NSMU,
                                        scalar2=None, op0=AL.add)
                DSZc = sc.tile([P, 2 * NS], F32, tag="DSZc")
                nc.vector.tensor_scalar(out=DSZc[:, 0:CO], in0=dxc,
                                        scalar1=NPHI2, scalar2=None,
                                        op0=AL.add)
                nc.vector.tensor_scalar(out=DSZc[:, CO:V], in0=dxc,
                                        scalar1=-1.0, scalar2=NPHI,
                                        op0=AL.mult, op1=AL.add)
                SFPc = sc.tile([P, 2], F32, tag="SFPc")
                nc.gpsimd.tensor_tensor(out=SFPc, in0=AIBI[:, 0:2],
                                        in1=albe_cn, op=AL.mult)
                nc.gpsimd.tensor_tensor(out=DSZc[:, V:V + 2], in0=SFPc,
                                        in1=RPs[:, 0:2], op=AL.subtract)
                nc.gpsimd.tensor_tensor(out=DSZc[:, V + 2:NS], in0=NRPs2,
                                        in1=SFPc[:, 1:2], op=AL.subtract)
                uvc = sc.tile([P, NS], F32, tag="uvc")
                nc.vector.tensor_tensor(out=uvc, in0=SZ[:, NS:2 * NS],
                                        in1=DSZc[:, 0:NS], op=AL.mult)
                vvc = sc.tile([P, NS], F32, tag="vvc")
                nc.vector.tensor_tensor(out=vvc, in0=uvc, in1=rszc,
                                        op=AL.add)
                nc.vector.scalar_tensor_tensor(
                    out=DSZc[:, NS:2 * NS], in0=vvc, scalar=-1.0,
                    in1=R[:, 0:NS], op0=AL.mult, op1=AL.mult)
                Qc = sc.tile([P, 2 * NS], F32, tag="Qc")
                nc.vector.scalar_tensor_tensor(
                    out=Qc, in0=DSZc, scalar=-1.0, in1=R,
                    op0=AL.mult, op1=AL.mult)
                qpc = sc.tile([P, 1], F32, tag="qpc")
                nc.vector.reduce_max(qpc, Qc, axis=AX)

                # --- [PE] T-c, [V] readout, [PE] B-c ---
                qrow_c = psq.tile([1, P], F32, tag="qrow")
                nc.tensor.transpose(qrow_c, qpc, IDENT)
                qm_c = sc.tile([1, 1], F32, tag="qm_c")
                nc.vector.reduce_max(qm_c, qrow_c, axis=AX)
                qc_c = sc.tile([1, 1], F32, tag="qc_c")
                nc.vector.tensor_scalar(out=qc_c, in0=qm_c, scalar1=1.0,
                                        scalar2=1.0 / 0.99, op0=AL.max,
                                        op1=AL.mult)
                acr = sc.tile([1, 1], F32, tag="acr")
                nc.vector.reciprocal(acr, qc_c)
                ALC = ps1.tile([P, 1], F32, tag="albc")
                nc.tensor.matmul(ALC, ONES[0:1, :], acr)

                # --- [V] state updates (0.99 pre-folded into ALC) ---
                OneM = sc.tile([P, 1], F32, tag="OneM")
                nc.vector.tensor_scalar(out=OneM, in0=ALC, scalar1=-1.0,
                                        scalar2=1.0, op0=AL.mult,
                                        op1=AL.add)
                nc.vector.scalar_tensor_tensor(
                    out=XT, in0=dxc, scalar=ALC, in1=XT,
                    op0=AL.mult, op1=AL.add)
                nc.vector.scalar_tensor_tensor(
                    out=SZ, in0=DSZc, scalar=ALC, in1=SZ,
                    op0=AL.mult, op1=AL.add)
                nc.vector.tensor_scalar(out=SZ, in0=SZ, scalar1=CLAMP,
                                        scalar2=None, op0=AL.max)

                # --- [V] phi-state update (all 8 scaled cols at once) ---
                nc.vector.tensor_scalar(out=PH, in0=PH, scalar1=OneM,
                                        scalar2=None, op0=AL.mult)

            # ---------------- end projection ----------------
            accF = sc.tile([P, 2], F32, tag="accF")
            fxv = sc.tile([P, CO], F32, tag="fxv")
            nc.vector.scalar_tensor_tensor(
                out=fxv, in0=XT, scalar=1.0, in1=F8,
                op0=AL.bypass, op1=AL.mult, accum_out=accF[:, 1:2])
            nc.vector.reduce_sum(accF[:, 0:1], XT, axis=AX)
            SXF = ps.tile([P, 2], F32, tag="pscr")   # [Sx|Fx]
            nc.tensor.matmul(SXF, ONES, accF)

            R2 = sc.tile([P, 2 * NS], F32, tag="R")
            nc.vector.reciprocal(R2, SZ)
            W2 = sc.tile([P, NS], F32, tag="W")
            nc.vector.tensor_tensor(out=W2, in0=SZ[:, NS:2 * NS],
                                    in1=R2[:, 0:NS], op=AL.mult)
            D2 = sc.tile([P, CO], F32, tag="Dt")
            nc.vector.scalar_tensor_tensor(
                out=D2, in0=W2[:, 0:CO], scalar=EPS, in1=W2[:, CO:V],
                op0=AL.add, op1=AL.add)
            DI2 = sc.tile([P, CO], F32, tag="DI")
            nc.vector.reciprocal(DI2, D2)
            nc.vector.tensor_scalar(out=DI2, in0=DI2, scalar1=1e-4,
                                    scalar2=None, op0=AL.max)
            acc2f = sc.tile([P, 2], F32, tag="accA")
            DIF2 = sc.tile([P, CO], F32, tag="DIF")
            nc.vector.scalar_tensor_tensor(
                out=DIF2, in0=DI2, scalar=1.0, in1=F8,
                op0=AL.bypass, op1=AL.mult, accum_out=acc2f[:, 0:1])
            nc.vector.reduce_sum(acc2f[:, 1:2], DI2, axis=AX)
            VUS2p = ps.tile([P, 2], F32, tag="pscr")  # [Sv|Su]
            nc.tensor.matmul(VUS2p, ONES, acc2f)
            VUS2 = sc.tile([P, 2], F32, tag="VUSS")
            nc.scalar.copy(VUS2, VUS2p)

            GT3 = sc.tile([P, 3], F32, tag="GT3")     # actives [g0 gf1 gf2]
            nc.vector.tensor_tensor(out=GT3, in0=z_s, in1=s_s, op=AL.is_gt)
            d0 = sc.tile([P, 1], F32, tag="d0")
            nc.vector.tensor_scalar(out=d0, in0=s_s[:, 0:1],
                                    scalar1=SXF[:, 0:1], scalar2=-C_CAP,
                                    op0=AL.add, op1=AL.add)
            dfa = sc.tile([P, 1], F32, tag="dfa")
            nc.vector.tensor_scalar(out=dfa, in0=s_s[:, 1:2],
                                    scalar1=SXF[:, 1:2], scalar2=HFA,
                                    op0=AL.add, op1=AL.subtract)
            dfb = sc.tile([P, 1], F32, tag="dfb")
            nc.vector.tensor_scalar(out=dfb, in0=s_s[:, 2:3],
                                    scalar1=SXF[:, 1:2], scalar2=HFB,
                                    op0=AL.subtract, op1=AL.add)
            ub = sc.tile([P, 1], F32, tag="ub")
            nc.vector.tensor_tensor(out=ub, in0=GT3[:, 2:3], in1=dfb,
                                    op=AL.mult)
            df = sc.tile([P, 1], F32, tag="df")
            nc.vector.scalar_tensor_tensor(
                out=df, in0=GT3[:, 1:2], scalar=dfa, in1=ub,
                op0=AL.mult, op1=AL.subtract)
            gf = sc.tile([P, 1], F32, tag="gf")
            nc.vector.tensor_tensor(out=gf, in0=GT3[:, 1:2],
                                    in1=GT3[:, 2:3], op=AL.max)
            nnum0 = sc.tile([P, 1], F32, tag="nnum0")
            nc.vector.scalar_tensor_tensor(
                out=nnum0, in0=gf, scalar=df, in1=d0,
                op0=AL.mult, op1=AL.subtract)
            nden0 = sc.tile([P, 1], F32, tag="nden0")
            nc.vector.scalar_tensor_tensor(
                out=nden0, in0=gf, scalar=VUS2[:, 0:1], in1=VUS2[:, 1:2],
                op0=AL.mult, op1=AL.subtract)
            ddt = sc.tile([P, 1], F32, tag="ddt")
            nc.vector.tensor_scalar(out=ddt, in0=nden0, scalar1=nden0,
                                    scalar2=TINY, op0=AL.mult, op1=AL.add)
            rdd = sc.tile([P, 1], F32, tag="rdd")
            nc.vector.reciprocal(rdd, ddt)
            v0a = sc.tile([P, 1], F32, tag="v0a")
            nc.vector.tensor_tensor(out=v0a, in0=nnum0, in1=nden0,
                                    op=AL.mult)
            v0b = sc.tile([P, 1], F32, tag="v0b")
            nc.vector.tensor_tensor(out=v0b, in0=v0a, in1=rdd, op=AL.mult)
            v0 = sc.tile([P, 1], F32, tag="v0")
            nc.vector.tensor_tensor(out=v0, in0=GT3[:, 0:1], in1=v0b,
                                    op=AL.mult)
            sv2t = sc.tile([P, 1], F32, tag="sv2t")
            nc.vector.tensor_scalar(out=sv2t, in0=VUS2[:, 0:1],
                                    scalar1=VUS2[:, 0:1], scalar2=TINY,
                                    op0=AL.mult, op1=AL.add)
            rsv = sc.tile([P, 1], F32, tag="rsv")
            nc.vector.reciprocal(rsv, sv2t)
            u1f = sc.tile([P, 1], F32, tag="u1f")
            nc.vector.tensor_tensor(out=u1f, in0=df, in1=VUS2[:, 0:1],
                                    op=AL.mult)
            v1a = sc.tile([P, 1], F32, tag="v1a")
            nc.vector.tensor_tensor(out=v1a, in0=u1f, in1=rsv, op=AL.mult)
            w1 = sc.tile([P, 1], F32, tag="w1")
            nc.vector.tensor_tensor(out=w1, in0=gf, in1=v1a, op=AL.mult)
            omgf = sc.tile([P, 1], F32, tag="omgf")
            nc.vector.tensor_scalar(out=omgf, in0=gf, scalar1=-1.0,
                                    scalar2=1.0, op0=AL.mult, op1=AL.add)
            w3 = sc.tile([P, 1], F32, tag="w3")
            nc.vector.tensor_tensor(out=w3, in0=omgf, in1=v0, op=AL.mult)
            v1 = sc.tile([P, 1], F32, tag="v1")
            nc.vector.tensor_tensor(out=v1, in0=w1, in1=w3, op=AL.add)
            bee = sc.tile([P, 1], F32, tag="bee")
            nc.vector.tensor_tensor(out=bee, in0=v1, in1=v0,
                                    op=AL.subtract)
            corr = sc.tile([P, CO], F32, tag="corr")
            nc.vector.tensor_scalar(out=corr, in0=F8, scalar1=bee,
                                    scalar2=v0, op0=AL.mult, op1=AL.add)
            mcor = sc.tile([P, CO], F32, tag="mcor")
            nc.vector.tensor_tensor(out=mcor, in0=DI2, in1=corr,
                                    op=AL.mult)
            nc.vector.tensor_tensor(out=XT, in0=XT, in1=mcor,
                                    op=AL.subtract)
            nc.vector.tensor_scalar(out=XT, in0=XT, scalar1=0.0,
                                    scalar2=1.0, op0=AL.max, op1=AL.min)

            nc.sync.dma_start(out=o_ap, in_=XT)

    return nc


_CACHE: dict = {}


def _get_nc():
    if "nc" not in _CACHE:
        nc = bacc.Bacc(None, target_bir_lowering=False)
        _build(nc)
        nc.finalize()
        _CACHE["nc"] = nc
    return _CACHE["nc"]


def kernel(x: np.ndarray, indices_male: np.ndarray) -> np.ndarray:
    nc = _get_nc()
    base = {
        "x": np.ascontiguousarray(x, dtype=np.float32),
        "ind": np.ascontiguousarray(indices_male, dtype=np.int32),
        "ones": np.ones((P, P), dtype=np.float32),
        "ident": np.eye(P, dtype=np.float32),
    }
    in_maps = [dict(base) for _ in range(8)]
    res = run_bass_kernel_spmd(nc, in_maps, core_ids=list(range(8)))
    return np.asarray(res.results[0]["out"], dtype=np.float32)


if __name__ == "__main__":
    rng = np.random.default_rng(0)
    x = rng.standard_normal((1, N)).astype(np.float32)
    f = (np.arange(N) % 2).astype(np.int32)
    out = kernel(x, f)
    print("out", out.shape, out.dtype, out[0, :6], out.sum())
